# revision 76
# baseline (speedup 1.0000x reference)
"""Blended-expert MLP (MoE routing) Trainium2 Bass kernel.

Math: reference computes, per layer,
    h = elu( einsum("bi,bio->bo", x, einsum("be,eio->bio", c, w)) + c @ b )
which factorizes as
    h = elu( sum_e (c[:,e] * x) @ W_e  +  c @ b )
(row-scaling commutes with the matmul), so per layer we scale X^T by
c_e on the vector engine (8 ops) and run 8 [rows,512]x[512,512]
matmuls plus one tiny K=8 matmul for the blended bias, ALL accumulating
into a single PSUM tile. Then ELU, then a PE transpose to produce the
next layer's stationary operand.

Sharding: data-parallel over the batch. B=512 rows split across 8
NeuronCores (64 rows each); the expert weights are replicated to every
core (fp16: 12 MB/core, fully SBUF-resident). No collectives (on-chip
AllReduce has a ~20us latency floor, worse than replication).

Layout per core:
  stationary operand = (c_e * X)^T chunks [128(i), 64(b)]
  moving operand     = W chunks  [128(i), 4096(e,o)] sliced per expert
  psum out           = [128(2 expert-halves x 64b), 512(o)], fp32

fp16 operands (not bf16): same DMA bytes and same 1-cycle/row matmul
rate, but 10 mantissa bits give ~7e-4 relative error vs ~6e-3.

Performance structure (measured ~50-54us on hardware, best 49.6us):
  ~7.5us fixed NEFF preamble | ~33.5us weight-DMA window (the wall:
  12 MB at ~360 GB/s/core, fully overlapped with compute) | ~4us
  compute tail | ~6us fixed all-engine end barrier.
Tricks that matter: PE warmup matmuls (HAM clock gate: cold PE runs at
1.2 GHz, warm 2.4 GHz), k-outer matmul order (each weight chunk's
matmuls fire on DMA arrival), even/odd expert pairs running
concurrently in the two column halves of the PE array (M=64 would
otherwise idle half the array), and a column-halved pipeline for the
psum-merge + ELU boundary chain.
"""

import numpy as np

B, E, D = 512, 8, 512
NCORES = 8
ROWS = B // NCORES  # 64
KC = D // 128  # 4 contraction chunks of 128

# pack tensor column layout (per 128 partitions)
PK_XT = 0  # [128, 256]: layer-1 x^T chunk k at cols [64k, 64k+64)
PK_CB = 256  # [128, 1024]: c broadcast; col 128e+64j+b = C[b,e], all partitions
PK_ID = PK_CB + E * 2 * ROWS  # [64, 64]: identity, partitions 0..63
PK_CT = PK_ID + ROWS  # [8, 64]: coef^T, partitions 0..7
PCK = PK_CT + ROWS

# matmul operand dtype: "f32" (exact, 4 cyc/row), "f32r" (fast fp32 mode,
# 1 cyc/row at N>=256), "bf16" (halves weight DMA, full-rate matmul).
MODE = "f16"

_NC_CACHE = {}


def _mmdt(mybir, mode):
    return {
        "f32": mybir.dt.float32,
        "f32r": mybir.dt.float32r,
        "bf16": mybir.dt.bfloat16,
        "f16": mybir.dt.float16,
    }[mode]


def _build(mode):
    from contextlib import ExitStack

    import concourse.bacc as bacc
    import concourse.mybir as mybir
    import concourse.tile as tile

    f32 = mybir.dt.float32
    mmdt = _mmdt(mybir, mode)
    Alu = mybir.AluOpType
    Act = mybir.ActivationFunctionType

    # Bacc (not raw Bass): its compile() legalizes the TRN2 one-sync-wait-
    # per-instruction limit by splitting excess waits into EventSemaphores
    nc = bacc.Bacc()
    pack_d = nc.declare_dram_parameter("pack", [128, PCK], mmdt, isOutput=False)
    bias_d = nc.declare_dram_parameter("biasd", [E, 3 * D], mmdt, isOutput=False)
    w_d = nc.declare_dram_parameter("w", [3, D, E * D], mmdt, isOutput=False)
    out_d = nc.declare_dram_parameter("out", [ROWS, D], f32, isOutput=True)

    with ExitStack() as ctx:
        tc = ctx.enter_context(tile.TileContext(nc))
        const = ctx.enter_context(tc.tile_pool(name="const", bufs=1))
        wpool = ctx.enter_context(
            tc.tile_pool(name="wp", bufs=12 if mode in ("bf16", "f16") else 8)
        )
        spool = ctx.enter_context(tc.tile_pool(name="sp", bufs=24))
        hpool = ctx.enter_context(tc.tile_pool(name="hp", bufs=2))
        xpool = ctx.enter_context(tc.tile_pool(name="xp", bufs=2))
        acc_ps = ctx.enter_context(tc.tile_pool(name="acc", bufs=3, space="PSUM"))
        pt_ps = ctx.enter_context(tc.tile_pool(name="pt", bufs=3, space="PSUM"))
        wm_ps = ctx.enter_context(tc.tile_pool(name="wm", bufs=1, space="PSUM"))

        # PE warmup: garbage matmuls on a zeroed tile (output never read),
        # emitted first so the HAM clock gate reaches 2.4 GHz before the
        # first real matmul (cold PE at 1.2 GHz otherwise doubles every
        # matmul). gpsimd is free right after its ~3.3us start preamble, so
        # it provides the earliest possible writer for the warm tile.
        warm = const.tile([128, ROWS + D], mmdt)
        nc.gpsimd.memset(warm[:], 0.0)
        wps = wm_ps.tile([ROWS, D], f32, tag="warm")
        for _ in range(14):
            nc.tensor.matmul(
                wps[:], warm[:, 0:ROWS], warm[:, ROWS:], start=True, stop=True
            )

        pack_t = const.tile([128, PCK], mmdt)
        pack_dma = nc.sync.dma_start(pack_t[:], pack_d[:])
        bias_t = const.tile([E, 3 * D], mmdt)
        nc.gpsimd.dma_start(bias_t[:], bias_d[:])

        coeft_ap = pack_t[0:E, PK_CT : PK_CT + ROWS]
        ident_ap = pack_t[0:ROWS, PK_ID : PK_ID + ROWS]
        xt_tile, xt_off = pack_t, PK_XT  # current x^T source: [128, 256] at offset

        # all weight-chunk DMAs up-front; the HWDGE lane round-robin plus
        # issue order paces them in consumption order at full bandwidth
        # (explicit chaining adds ~2us completion-latency per hop - worse)
        all_wts = []
        for layer in range(3):
            for k in range(KC):
                wt = wpool.tile([128, E * D], mmdt, tag="w")
                rs_ = slice(128 * k, 128 * (k + 1))
                if layer == 2 and k == KC - 1:
                    # split the final chunk: its first half (experts 0-3)
                    # lands ~1.2us earlier, so only experts 4-7's last
                    # matmuls gate on the very last 0.5 MB transfer
                    nc.sync.dma_start(
                        wt[:, 0 : E * D // 2], w_d[layer, rs_, 0 : E * D // 2]
                    )
                    nc.sync.dma_start(
                        wt[:, E * D // 2 :], w_d[layer, rs_, E * D // 2 :]
                    )
                else:
                    nc.sync.dma_start(wt[:], w_d[layer, rs_, :])
                all_wts.append(wt)

        for layer in range(3):
            wts = all_wts[layer * KC : (layer + 1) * KC]

            # scale x^T by c_e along the batch (free) dim: one DVE op per
            # expert over all 4 chunks at once
            # per-chunk rescale: TT(e,k) gates only on evacuation k of the
            # previous layer's transpose, and matmul (e,k) gates only on
            # TT(e,k) (subtile column tracking), so the boundary pipelines
            # at chunk granularity. c-broadcast is stored once (64 cols/e).
            # per chunk-pair rescale: TT(e,half) gates on the first/last two
            # transpose evacuations only, and matmul (e,k) gates on its half
            # (subtile column tracking)
            scaled = []
            for e in range(E):
                sc = spool.tile([128, KC * ROWS], mmdt, tag="sc")
                for half in range(2):
                    lo, hi = 2 * ROWS * half, 2 * ROWS * (half + 1)
                    nc.vector.tensor_tensor(
                        out=sc[:, lo:hi],
                        in0=xt_tile[:, xt_off + lo : xt_off + hi],
                        in1=pack_t[
                            :,
                            PK_CB + 2 * ROWS * e : PK_CB + 2 * ROWS * (e + 1),
                        ],
                        op=Alu.mult,
                    )
                scaled.append(sc)

            # one accumulation group: 32 expert matmuls + bias matmul (K=8).
            # k-outer order: each weight chunk's 8 expert matmuls fire as
            # soon as that chunk's DMA lands, overlapping the next transfer.
            # Even/odd experts run CONCURRENTLY in the two column halves of
            # the PE array (tile_position), since M=64 only fills half the
            # array; the partition halves of acc are summed afterwards.
            acc = acc_ps.tile([2 * ROWS, D], f32, tag="acc")
            # bias matmul opens the even-half group so the even half is done
            # (and can evacuate) while the last odd matmuls still run
            nc.tensor.matmul(
                acc[0:ROWS, :],
                coeft_ap,
                bias_t[:, D * layer : D * (layer + 1)],
                start=True,
                stop=False,
                tile_position=(0, 0),
                skip_group_check=True,
            )
            for k in range(KC):
                for e in range(E):
                    half = e % 2
                    nc.tensor.matmul(
                        acc[half * ROWS : (half + 1) * ROWS, :],
                        scaled[e][:, ROWS * k : ROWS * (k + 1)],
                        wts[k][:, D * e : D * (e + 1)],
                        start=(k == 0 and e == 1),
                        stop=(k == KC - 1 and e >= E - 2),
                        tile_position=(0, half * ROWS),
                        skip_group_check=True,
                    )
            # evacuate even half (ACT) + merge halves (DVE) + elu + transpose,
            # pipelined per 128-column quarter: transpose k consumes exactly
            # quarter k, so each quarter flows through the whole boundary
            # chain independently
            t0 = hpool.tile([ROWS, D], f32, tag="t0")
            hpre = hpool.tile([ROWS, D], f32, tag="hpre")
            HD = D // 2
            if layer < 2:
                # keep the PE clock warm across the elu/transpose boundary
                # (a >3.4us PE-idle window would re-throttle to 1.2 GHz)
                for _ in range(8):
                    nc.tensor.matmul(
                        wps[:], warm[:, 0:ROWS], warm[:, ROWS:],
                        start=True, stop=True,
                    )

            if layer < 2:
                # per quarter q: copy+merge, elu(x)=max(x,0)+min(exp(x)-1,0),
                # then transpose + evacuation - all stages pipeline across
                # quarters on alternating engines
                ex = hpool.tile([ROWS, D], f32, tag="ex")
                h = hpool.tile([ROWS, D], mmdt, tag="h")
                xt_t = xpool.tile([128, KC * ROWS], mmdt, tag="xt")
                for q in range(KC):
                    qs = slice(128 * q, 128 * (q + 1))
                    nc.scalar.copy(t0[:, qs], acc[0:ROWS, qs])
                    nc.vector.tensor_tensor(
                        out=hpre[:, qs], in0=t0[:, qs], in1=acc[ROWS:, qs],
                        op=Alu.add,
                    )
                    nc.scalar.activation(ex[:, qs], hpre[:, qs], Act.Exp)
                    nc.vector.tensor_scalar(
                        ex[:, qs], ex[:, qs], 1.0, 0.0, Alu.subtract, Alu.min
                    )
                    nc.vector.scalar_tensor_tensor(
                        out=h[:, qs],
                        in0=hpre[:, qs],
                        scalar=0.0,
                        in1=ex[:, qs],
                        op0=Alu.max,
                        op1=Alu.add,
                    )
                    pt = pt_ps.tile([128, ROWS], mmdt, tag="pt")
                    nc.tensor.transpose(pt[:], h[:, qs], ident_ap)
                    dst = xt_t[:, ROWS * q : ROWS * (q + 1)]
                    if q % 2 == 0:
                        nc.scalar.copy(dst, pt[:])
                    else:
                        nc.vector.tensor_copy(dst, pt[:])
                xt_tile, xt_off = xt_t, 0
            else:
                # stream the output per column half, right behind the merge;
                # separate staging tiles per half so the second ACT copy
                # doesn't serialize behind the first DVE merge (t0 WAR)
                for c in range(2):
                    cs = slice(HD * c, HD * (c + 1))
                    t0c = hpool.tile(
                        [ROWS, HD], f32, tag="t0a" if c == 0 else "t0b"
                    )
                    nc.scalar.copy(t0c[:], acc[0:ROWS, cs])
                    nc.vector.tensor_tensor(
                        out=hpre[:, cs], in0=t0c[:], in1=acc[ROWS:, cs],
                        op=Alu.add,
                    )
                    nc.sync.dma_start(out_d[:, cs], hpre[:, cs])

    nc.compile()
    return nc


def _get_nc(mode):
    if mode not in _NC_CACHE:
        _NC_CACHE[mode] = _build(mode)
    return _NC_CACHE[mode]


def _prep_in_maps(inputs, mode):
    import ml_dtypes

    X = np.asarray(inputs["X"], np.float32)
    C = np.asarray(inputs["blending_coef"], np.float32)
    ws = [np.asarray(inputs[f"w_l{i}"], np.float32) for i in (1, 2, 3)]
    bs = [np.asarray(inputs[f"b_l{i}"], np.float32) for i in (1, 2, 3)]

    mm_np = {
        "f32": np.float32,
        "f32r": np.float32,
        "bf16": ml_dtypes.bfloat16,
        "f16": np.float16,
    }[mode]

    # W[l][i, e*D+o] = w_l[e, i, o]
    W = np.stack([w.transpose(1, 0, 2).reshape(D, E * D) for w in ws]).astype(mm_np)
    Bb = np.concatenate(bs, axis=1).astype(mm_np)  # [E, 3*D]

    in_maps = []
    for c in range(NCORES):
        rs = slice(c * ROWS, (c + 1) * ROWS)
        pack = np.zeros((128, PCK), np.float32)
        # xt chunks: pack[p, 64k+b] = X[rows][b, 128k+p]
        xt = np.ascontiguousarray(X[rs].T)  # [512, 64]
        pack[:, PK_XT : PK_XT + KC * ROWS] = (
            xt.reshape(KC, 128, ROWS).transpose(1, 0, 2).reshape(128, KC * ROWS)
        )
        # c broadcast: pack[p, PK_CB + 128e + 64j + b] = C[rs][b, e]
        pack[:, PK_CB : PK_CB + E * 2 * ROWS] = np.broadcast_to(
            C[rs].T[:, None, :], (E, 2, ROWS)
        ).reshape(1, E * 2 * ROWS)
        pack[0:ROWS, PK_ID : PK_ID + ROWS] = np.eye(ROWS, dtype=np.float32)
        pack[0:E, PK_CT : PK_CT + ROWS] = C[rs].T
        in_maps.append({"pack": pack.astype(mm_np), "biasd": Bb, "w": W})
    return in_maps


def run(inputs, mode=MODE, trace=False):
    """Returns (output [512,512] fp32, BassKernelResults)."""
    from concourse.bass_utils import run_bass_kernel_spmd

    nc = _get_nc(mode)
    in_maps = _prep_in_maps(inputs, mode)
    res = run_bass_kernel_spmd(nc, in_maps, list(range(NCORES)), trace=trace)
    out = np.concatenate([r["out"] for r in res.results], axis=0)
    return out, res


def kernel(**inputs) -> np.ndarray:
    out, _ = run(inputs)
    return out
